# revision 42
# baseline (speedup 1.0000x reference)
"""Llama GQA attention layer (T=2048, H=4096, 32 q heads / 8 kv heads, hd=128),
tensor-parallel over heads across 8 Trainium2 NeuronCores.

Per core c: 4 q heads + 1 kv head (wq/wk/wv column slices, wo row slice).
Each core computes a full [T, H] partial o_proj output (bf16); partials are
summed on host (the all-reduce of the TP scheme).

Mixed precision scheme (error budget is dominated by the early causal rows,
which have near-delta attention and O(1)-magnitude outputs):
- t-chunk 0 of QKV, and q-chunk 0 of attention+o_proj run in bf16.
- everything else runs fp8e4(m3) with MatmulPerfMode.DoubleRow (2 fp8
  weights/PE cell, 2 MACs/cycle, measured same 216ns/MM as bf16 at half
  the instruction count): QKV for t-chunks 1-3 contracts hidden-dim pairs,
  PV+softmax-denominator and o_proj for q-chunks 1-3 contract
  key-chunk/head pairs. Those rows average >500 keys, so fp8 noise washes
  out by ~1/sqrt(k_eff) against a 40x larger error budget.
- exp() carries a built-in bias of -ln(16) so probabilities fit fp8e4
  range; the factor cancels between numerator and denominator.

DMA discipline: every dma_start costs ~600ns on the issuing engine queue
(HWDGE descriptor generation), so transfers are batched aggressively:
weights land as one combined [128, 2, 768] (q|k|v) tile per hidden
pair-chunk, hiddenT tiles are loaded two chunks per DMA, RoPE half-swaps
move all 5 heads in two DMAs, and o_proj stores 4 output chunks per DMA.
"""

import sys

if "/opt/trn_rl_repo" not in sys.path:
    sys.path.insert(0, "/opt/trn_rl_repo")

import numpy as np

import concourse.bass as bass
import concourse.bacc as bacc
import concourse.tile as tile
import concourse.mybir as mybir
from concourse import bass_utils

T = 2048
H = 4096
NQ = 32
NKV = 8
HD = 128
THETA = 10000.0
N_CORES = 8
NH = NQ // N_CORES          # local q heads per core
HALF = HD // 2
TC = 512                    # t-chunk (matmul free dim)
NTC = T // TC               # 4
NKCH = H // 128             # 32 hidden chunks
NPCH = H // 256             # 16 hidden pair-chunks (DoubleRow)
WALL = NH * HD + 2 * HD     # 768: combined q|k|v weight columns
SCALE = float(HD) ** -0.5
EXPB = 2.772588722239781    # ln(16): exp bias so fp8 probs stay in range

F32 = mybir.dt.float32
F32R = mybir.dt.float32r
BF16 = mybir.dt.bfloat16
F8 = mybir.dt.float8e4
ALU = mybir.AluOpType
ACTF = mybir.ActivationFunctionType
DR = mybir.MatmulPerfMode.DoubleRow


def _build():
    nc = bacc.Bacc("TRN2", target_bir_lowering=False, debug=False,
                   num_devices=N_CORES)
    # t-chunk 0 hiddenT pair tiles, bf16: [kp][ki, ko, t] = hT[256kp+128ko+ki, t]
    ht0 = nc.dram_tensor("ht0", [NPCH, 128, 2, TC], BF16,
                         kind="ExternalInput").ap()
    # t-chunks 1-3 hiddenT double-pair tiles, fp8:
    # [(t-1)*8+pq][ki, 2*(pc%2)+ko, t'] = hT[256pc+128ko+ki, 512t+t']
    ht8 = nc.dram_tensor("ht8", [3 * NPCH // 2, 128, 4, TC], F8,
                         kind="ExternalInput").ap()
    # combined bf16 weight pairs [kp][ki, ko, q0..q3|k|v]
    w0all = nc.dram_tensor("w0all", [NKCH // 2, 128, 2, WALL], BF16,
                           kind="ExternalInput").ap()
    # combined fp8 pair weights [pc][ki, ko, q0..q3|k|v]
    w8all = nc.dram_tensor("w8all", [NPCH, 128, 2, WALL], F8,
                           kind="ExternalInput").ap()
    wob = nc.dram_tensor("wob", [NH * HD, H], BF16, kind="ExternalInput").ap()
    wo8 = nc.dram_tensor("wo8", [NH // 2, 128, 2, H], F8,
                         kind="ExternalInput").ap()
    # cos | sin, sign-folded, bf16
    cossin = nc.dram_tensor("cossin", [HD, 2 * T], BF16,
                            kind="ExternalInput").ap()
    # identity | causal-triangle, bf16
    bmisc = nc.dram_tensor("bmisc", [128, 256], BF16,
                           kind="ExternalInput").ap()
    tri8 = nc.dram_tensor("tri8", [128, 128], F8, kind="ExternalInput").ap()
    ones_r = nc.dram_tensor("ones_r", [1, 128], F32R,
                            kind="ExternalInput").ap()
    # per q-chunk: [qc][ki, mo*TC + t']
    out_t = nc.dram_tensor("out_t", [NTC, 128, (H // 128) * TC], BF16,
                           kind="ExternalOutput").ap()

    with tile.TileContext(nc) as tc:
        _body(tc, ht0, ht8, w0all, w8all, wob, wo8, cossin, bmisc, tri8,
              ones_r, out_t)
    nc.compile()
    return nc


def _body(tc, ht0, ht8, w0all, w8all, wob, wo8, cossin, bmisc, tri8,
          ones_r, out_t):
    nc = tc.nc

    with (
        tc.tile_pool(name="persist", bufs=1) as persist,
        tc.tile_pool(name="small", bufs=1) as small,
        tc.tile_pool(name="p0p", bufs=NH * 4) as p0p,
    ):
        # live across all phases
        qT = persist.tile([128, NH * T], BF16, tag="qT")     # [d, h*T + t]
        kT = persist.tile([128, T], BF16, tag="kT")          # [d, t]
        v_sb = persist.tile([128, T], BF16, tag="v")         # chunk i: v[t,d]
        v8 = persist.tile([128, NTC * 4, 128], F8, tag="v8")  # [t, chunk, d]
        bmisc_sb = small.tile([128, 256], BF16, tag="bmisc")
        tri8_sb = small.tile([128, 128], F8, tag="tri8")
        onesb_sb = small.tile([128, 1], BF16, tag="onesb")
        ones8_sb = small.tile([128, 2, 16], F8, tag="ones8")
        onesr_sb = small.tile([1, 128], F32R, tag="onesr")
        ebias_sb = small.tile([128, 1], F32, tag="ebias")
        nc.vector.memset(onesb_sb[:], 1.0)
        nc.vector.memset(ones8_sb[:], 1.0)
        nc.vector.memset(ebias_sb[:], -EXPB)
        ident_sb = bmisc_sb[:, 0:128]
        trib_sb = bmisc_sb[:, 128:256]

        qc0_ps = []  # q-chunk 0 probabilities, produced during phase 1
        # ---------------- phase 1: QKV projections + RoPE + V transpose ----
        with (
            tc.tile_pool(name="ph1w", bufs=1) as ph1w,
            tc.tile_pool(name="ph1", bufs=6) as ph1,
            tc.tile_pool(name="rope", bufs=2) as rope,
            tc.tile_pool(name="ps1", bufs=1, space="PSUM") as ps1,
            tc.tile_pool(name="pst", bufs=1, space="PSUM") as pst,
            tc.tile_pool(name="ps_s0", bufs=1, space="PSUM") as ps_s0,
        ):
            csb = ph1w.tile([128, 2 * T], BF16, tag="csb")
            vT_sb = ph1w.tile([128, T], BF16, tag="vT")
            w0_t, w8_t = [], []

            def _transp(t):
                # V has no rope; transpose vT to natural [t, d] + fp8 copy
                for i in range(t * 4, t * 4 + 4):
                    tp = pst.tile([128, 128], BF16, tag="tp")
                    nc.tensor.transpose(
                        tp[:], vT_sb[:, i * 128:(i + 1) * 128], ident_sb)
                    nc.scalar.copy(v_sb[:, i * 128:(i + 1) * 128], tp[:])
                    nc.vector.tensor_copy(v8[:, i, :],
                                          v_sb[:, i * 128:(i + 1) * 128])

            def _rope(t, qps, kps, vps):
                # RoPE: out = x*cos2 + swap(x)*sin2 (swap = halves exchanged).
                # k first, then head 0, then v, so q-chunk 0's attention can
                # start as early as possible after the t=0 pass.
                raw5 = rope.tile([128, 5 * TC], BF16, tag="raw5")
                sw5 = rope.tile([128, 5 * TC], BF16, tag="sw5")
                cosv = csb[:, t * TC:(t + 1) * TC]
                sinv = csb[:, T + t * TC: T + (t + 1) * TC]
                for step, hc in enumerate((NH, 0, 1, 2, 3)):
                    src_ps = qps[hc] if hc < NH else kps
                    sl = slice(hc * TC, (hc + 1) * TC)
                    nc.scalar.copy(raw5[:, sl], src_ps[:])
                    # gpsimd SWDGE keeps the half-swaps off the Sync queue
                    # so the next t-chunk's ht/weight loads aren't blocked
                    nc.gpsimd.dma_start(sw5[0:HALF, sl], raw5[HALF:128, sl])
                    nc.gpsimd.dma_start(sw5[HALF:128, sl], raw5[0:HALF, sl])
                    dst = (qT[:, hc * T + t * TC: hc * T + (t + 1) * TC]
                           if hc < NH else kT[:, t * TC:(t + 1) * TC])
                    a = rope.tile([128, TC], BF16, tag="ra")
                    b = rope.tile([128, TC], BF16, tag="rb16")
                    nc.vector.tensor_tensor(a[:], raw5[:, sl], cosv, ALU.mult)
                    nc.vector.tensor_tensor(b[:], sw5[:, sl], sinv, ALU.mult)
                    nc.vector.tensor_tensor(dst, a[:], b[:], ALU.add)
                    if step == 1:
                        nc.scalar.copy(vT_sb[:, t * TC:(t + 1) * TC], vps[:])
                        _transp(t)

            def _qc0_scores():
                # q-chunk 0 scores+exp overlap the t=1..3 QKV stream: they
                # only need t-chunk 0's rope output, and the exp chain hides
                # under ~60us of DoubleRow matmuls instead of padding phase 2
                for h in range(NH):
                    qh = qT[:, h * T: h * T + TC]
                    for kc in range(4):
                        off = 128 * kc      # all qc=0 chunks diagonal
                        sT = ps_s0.tile([128, TC], F32, tag="sT0")
                        nc.tensor.matmul(
                            sT[:, off:], kT[:, kc * 128:(kc + 1) * 128],
                            qh[:, off:], start=True, stop=True)
                        p = p0p.tile([128, TC], BF16, tag="p0", name="p0")
                        nc.scalar.activation(p[:, off:], sT[:, off:],
                                             ACTF.Exp, scale=SCALE,
                                             bias=ebias_sb[:])
                        if off:
                            nc.gpsimd.memset(p[:, 0:off], 0.0)
                        nc.gpsimd.tensor_tensor(
                            p[:, off:off + 128], p[:, off:off + 128],
                            trib_sb, ALU.mult)
                        qc0_ps.append(p)

            for t in (0, 1, 2, 3):
                qps = [ps1.tile([128, TC], F32, tag=f"qps{fc}",
                                name=f"qps{fc}") for fc in range(NH)]
                kps = ps1.tile([128, TC], F32, tag="kps")
                vps = ps1.tile([128, TC], F32, tag="vps")
                if t == 0:
                    for k in range(NKCH):
                        if k % 2 == 0:
                            w0t = ph1w.tile([128, 2, WALL], BF16,
                                            tag=f"w0a{k // 2}",
                                            name=f"w0a{k // 2}")
                            if k == 0:
                                # k/v slice first: the opening matmul only
                                # waits on 64KB instead of 384KB
                                nc.sync.dma_start(w0t[:, 0, 512:768],
                                                  w0all[0][:, 0, 512:768])
                                nc.sync.dma_start(w0t[:, 0, 0:512],
                                                  w0all[0][:, 0, 0:512])
                                nc.sync.dma_start(w0t[:, 1, :],
                                                  w0all[0][:, 1, :])
                            else:
                                nc.sync.dma_start(w0t[:], w0all[k // 2])
                            w0_t.append(w0t)
                        if k == 4:
                            nc.sync.dma_start(bmisc_sb[:], bmisc[:, :])
                        if k == 5:
                            nc.sync.dma_start(tri8_sb[:], tri8[:, :])
                        if k == 8:
                            nc.sync.dma_start(csb[:], cossin[:, :])
                        if k % 2 == 0:
                            htt = ph1.tile([128, 2, TC], BF16, tag="ht0")
                            if k == 0:
                                # split the very first load so MM #1 only
                                # waits on half the bytes
                                nc.sync.dma_start(htt[:, 0, :],
                                                  ht0[0][:, 0, :])
                                nc.sync.dma_start(htt[:, 1, :],
                                                  ht0[0][:, 1, :])
                            else:
                                nc.sync.dma_start(htt[:], ht0[k // 2])
                        mv = htt[:, k % 2, :]
                        st, sp = (k == 0), (k == NKCH - 1)
                        wsl = w0_t[k // 2]
                        # k/v first: kps completes earlier, shortening the
                        # RoPE-k chain that gates q-chunk 0's first scores
                        nc.tensor.matmul(kps[:], wsl[:, k % 2, 512:640], mv,
                                         start=st, stop=sp)
                        nc.tensor.matmul(vps[:], wsl[:, k % 2, 640:768], mv,
                                         start=st, stop=sp)
                        for fc in range(NH):
                            nc.tensor.matmul(
                                qps[fc][:],
                                wsl[:, k % 2, fc * 128:(fc + 1) * 128],
                                mv, start=st, stop=sp)
                else:
                    for pc in range(NPCH):
                        if t == 1:
                            w8t = ph1w.tile([128, 2, WALL], F8,
                                            tag=f"w8a{pc}", name=f"w8a{pc}")
                            nc.sync.dma_start(w8t[:], w8all[pc])
                            w8_t.append(w8t)
                        if pc % 2 == 0:
                            htt = ph1.tile([128, 4, TC], F8, tag="ht8")
                            nc.sync.dma_start(
                                htt[:], ht8[(t - 1) * (NPCH // 2) + pc // 2])
                        mv = htt[:, 2 * (pc % 2):2 * (pc % 2) + 2, :]
                        st, sp = (pc == 0), (pc == NPCH - 1)
                        for fc in range(NH):
                            nc.tensor.matmul(
                                qps[fc][:],
                                w8_t[pc][:, :, fc * 128:(fc + 1) * 128],
                                mv, start=st, stop=sp, perf_mode=DR)
                        nc.tensor.matmul(
                            kps[:], w8_t[pc][:, :, 512:640], mv,
                            start=st, stop=sp, perf_mode=DR)
                        nc.tensor.matmul(
                            vps[:], w8_t[pc][:, :, 640:768], mv,
                            start=st, stop=sp, perf_mode=DR)
                _rope(t, qps, kps, vps)
                if t == 0:
                    _qc0_scores()

        # ------- phase 2+3: attention interleaved with o_proj, per q-chunk --
        with (
            tc.tile_pool(name="late", bufs=1) as late,
            tc.tile_pool(name="attq", bufs=2) as attq,
            tc.tile_pool(name="att8", bufs=2) as att8,
            tc.tile_pool(name="p8p", bufs=6) as p8p,
            tc.tile_pool(name="pbp", bufs=4) as pbp,
            tc.tile_pool(name="obp", bufs=4) as obp,
            tc.tile_pool(name="rbp", bufs=8) as rbp,
            tc.tile_pool(name="ps_s", bufs=3, space="PSUM") as ps_s,
            tc.tile_pool(name="ps_o", bufs=1, space="PSUM") as ps_o,
            tc.tile_pool(name="ps_d", bufs=1, space="PSUM") as ps_d,
            tc.tile_pool(name="ps_op", bufs=2, space="PSUM") as ps_op,
            tc.tile_pool(name="ps_rb", bufs=1, space="PSUM") as ps_rb,
        ):
            attn_n = late.tile([128, NH * TC], BF16, tag="attn_n")
            nc.sync.dma_start(onesr_sb[:], ones_r[:, :])
            wob_t, wo8_t = [], []
            for h in range(NH):
                wot = late.tile([128, H], BF16, tag=f"wob{h}", name=f"wob{h}")
                nc.sync.dma_start(wot[:], wob[h * 128:(h + 1) * 128, :])
                wob_t.append(wot)
            for pr in range(NH // 2):
                wo8t = late.tile([128, 2, H], F8, tag=f"wo8{pr}",
                                 name=f"wo8{pr}")
                nc.sync.dma_start(wo8t[:], wo8[pr])
                wo8_t.append(wo8t)

            def _norm(qc, attn_t, rcs):
                """normalize by 1/den, broadcast [1,TC] across partitions via
                a rank-1 PE matmul (ones_col x recip_row) into PSUM; write
                bf16 (qc=0) or fp8 head-slabs (qc>=1) for o_proj."""
                a8 = None
                if qc > 0:
                    a8 = att8.tile([128, NH, TC], F8, tag="attn8",
                                   name="attn8")
                for h, rcr in enumerate(rcs):
                    rb = ps_rb.tile([128, TC], F32, tag="rbps", name="rbps")
                    nc.tensor.matmul(rb[:], onesr_sb[:], rcr[:],
                                     start=True, stop=True)
                    dst = (attn_n[:, h * TC:(h + 1) * TC] if qc == 0
                           else a8[:, h, :])
                    nc.vector.tensor_tensor(
                        dst, attn_t[:, h * TC:(h + 1) * TC], rb[:], ALU.mult)
                return a8

            def _oproj(qc, a8):
                """out_t[qc][mo-part, mo*TC+t] = sum_f wo[f, mo] * attnT[f, t];
                4 mo-chunks per store DMA, drains alternate Scalar/Vector."""
                ob4 = None
                for mo in range(H // 128):
                    op = ps_op.tile([128, TC], F32, tag="op", name="op")
                    if qc == 0:
                        for h in range(NH):
                            nc.tensor.matmul(
                                op[:],
                                wob_t[h][:, mo * 128:(mo + 1) * 128],
                                attn_n[:, h * TC:(h + 1) * TC],
                                start=(h == 0), stop=(h == NH - 1))
                    else:
                        for pr in range(NH // 2):
                            nc.tensor.matmul(
                                op[:],
                                wo8_t[pr][:, :, mo * 128:(mo + 1) * 128],
                                a8[:, 2 * pr:2 * pr + 2, :],
                                start=(pr == 0), stop=(pr == NH // 2 - 1),
                                perf_mode=DR)
                    if mo % 4 == 0:
                        ob4 = obp.tile([128, 4 * TC], BF16, tag="ob4",
                                       name="ob4")
                    dst = ob4[:, (mo % 4) * TC:(mo % 4 + 1) * TC]
                    nc.vector.tensor_copy(dst, op[:])
                    if mo % 4 == 3:
                        nc.sync.dma_start(
                            out_t[qc][:, (mo - 3) * TC:(mo + 1) * TC], ob4[:])

            norm_pending = []
            for qc in range(NTC):
                nkc = (qc + 1) * (TC // 128)     # causal k chunks of 128
                attn_t = attq.tile([128, NH * TC], BF16, tag="attnq",
                                   name="attnq")
                rcs = []
                for h in range(NH):
                    qh = qT[:, h * T + qc * TC: h * T + (qc + 1) * TC]
                    po = ps_o.tile([128, TC], F32, tag="po")
                    pd = ps_d.tile([1, TC], F32, tag="pd")
                    if qc == 0:
                        ps = qc0_ps[h * 4:(h + 1) * 4]

                        def _pv0(kc, nkc=nkc, po=po, pd=pd, ps=ps):
                            st, sp = (kc == 0), (kc == nkc - 1)
                            o = 128 * kc        # masked-zero column prefix
                            nc.tensor.matmul(
                                po[:, o:], v_sb[:, kc * 128:(kc + 1) * 128],
                                ps[kc][:, o:], start=st, stop=sp,
                                skip_group_check=True)
                            nc.tensor.matmul(pd[:, o:], onesb_sb[:],
                                             ps[kc][:, o:],
                                             start=st, stop=sp,
                                             skip_group_check=True)

                        for kc in range(nkc):
                            _pv0(kc)
                    else:
                        npair = nkc // 2
                        prs = []

                        def _pvp(r, npair=npair, nkc=nkc, po=po, pd=pd,
                                 prs=prs):
                            st, sp = (r == 0), (r == npair - 1)
                            # masked-zero column prefix of the even chunk
                            o = max(0, 128 * (2 * r - (nkc - 4)))
                            nc.tensor.matmul(
                                po[:, o:], v8[:, 2 * r:2 * r + 2, :],
                                prs[r][:, :, o:], start=st, stop=sp,
                                perf_mode=DR, skip_group_check=True)
                            nc.tensor.matmul(
                                pd[:, o:], ones8_sb[:, :, 0:1],
                                prs[r][:, :, o:], start=st, stop=sp,
                                perf_mode=DR, skip_group_check=True)

                        for kc in range(nkc):
                            di = kc - (nkc - 4)
                            off = 128 * di if di >= 0 else 0
                            slab = kc & 1
                            if slab == 0:
                                p8t = p8p.tile([128, 2, TC], F8, tag="p8",
                                               name="p8")
                                prs.append(p8t)
                            else:
                                p8t = prs[-1]
                            sT = ps_s.tile([128, TC], F32, tag="sT")
                            nc.tensor.matmul(
                                sT[:, off:], kT[:, kc * 128:(kc + 1) * 128],
                                qh[:, off:], start=True, stop=True)
                            nc.scalar.activation(p8t[:, slab, off:],
                                                 sT[:, off:], ACTF.Exp,
                                                 scale=SCALE,
                                                 bias=ebias_sb[:])
                            if off:
                                nc.gpsimd.memset(p8t[:, slab, 0:off], 0.0)
                            if di >= 0:
                                nc.gpsimd.tensor_tensor(
                                    p8t[:, slab, off:off + 128],
                                    p8t[:, slab, off:off + 128],
                                    tri8_sb[:], ALU.mult)
                            # PV trails the score stream by one pair so the
                            # PE never waits on the exp
                            if slab == 1 and len(prs) >= 2:
                                _pvp(len(prs) - 2)
                        _pvp(npair - 1)
                    # drain PSUM fast; normalization happens in _norm, off
                    # the PE critical path
                    nc.vector.tensor_copy(
                        attn_t[:, h * TC:(h + 1) * TC], po[:])
                    rc = rbp.tile([1, TC], F32, tag="rc")
                    nc.vector.reciprocal_approx_fast(out=rc[:], in_=pd[:])
                    rcr = rbp.tile([1, TC], F32R, tag="rcr")
                    nc.vector.tensor_copy(rcr[:], rc[:])
                    rcs.append(rcr)
                norm_pending.append((qc, attn_t, rcs))

                # normalize + o_proj pipelined one q-chunk behind attention so
                # the reciprocal chain never stalls the PE stream
                if qc >= 1:
                    pqc, pattn, prcs = norm_pending.pop(0)
                    a8 = _norm(pqc, pattn, prcs)
                    _oproj(pqc, a8)
            pqc, pattn, prcs = norm_pending.pop(0)
            a8 = _norm(pqc, pattn, prcs)
            _oproj(pqc, a8)


_NC = None
LAST_EXEC_NS = None
LAST_TRACE = None
LAST_INSTS = None


def _ensure_profile_hook():
    """Register the axon NTFF profiling hook (container lacks antenv.axon_hooks)."""
    import types
    import antenv
    if "antenv.axon_hooks" in sys.modules:
        return
    hooks_mod = types.ModuleType("antenv.axon_hooks")
    _h = [None]
    hooks_mod.set_axon_ntff_profile_hook = lambda hk: _h.__setitem__(0, hk)
    hooks_mod.get_axon_ntff_profile_hook = lambda: _h[0]
    sys.modules["antenv.axon_hooks"] = hooks_mod
    antenv.axon_hooks = hooks_mod
    from trn_agent_boot.trn_boot import _ntff_profile_via_ctypes
    hooks_mod.set_axon_ntff_profile_hook(
        _ntff_profile_via_ctypes("/opt/axon/libaxon_pjrt.so"))
    bass_utils.upload_artifacts = lambda tmpdir: "local://skipped"


def kernel(positions, hidden_states, wq, wk, wv, wo, _trace=False, **_unused):
    global _NC, LAST_EXEC_NS, LAST_TRACE, LAST_INSTS
    import ml_dtypes
    BF = ml_dtypes.bfloat16
    E4 = ml_dtypes.float8_e4m3

    positions = np.asarray(positions)
    hidden_states = np.asarray(hidden_states, dtype=np.float32)
    wq = np.asarray(wq, dtype=np.float32)
    wk = np.asarray(wk, dtype=np.float32)
    wv = np.asarray(wv, dtype=np.float32)
    wo = np.asarray(wo, dtype=np.float32)

    # host-side input prep (sharding + layout)
    hT = np.ascontiguousarray(hidden_states.T)                      # [H, T]
    # [kp, ko, ki, t] from hT[256kp + 128ko + ki, t]
    ht0_np = np.ascontiguousarray(
        hT[:, 0:TC].reshape(NPCH, 2, 128, TC)
        .transpose(0, 2, 1, 3)).astype(BF)
    # [pc, ko, ki, tc, t] -> [tc, pc, ki, ko, t], tc 1..3, then group pc pairs
    x8 = (hT.reshape(NPCH, 2, 128, NTC, TC).transpose(3, 0, 2, 1, 4)[1:]
          .reshape(3, NPCH // 2, 2, 128, 2, TC).transpose(0, 1, 3, 2, 4, 5))
    ht8_np = np.ascontiguousarray(
        x8.reshape(3 * NPCH // 2, 128, 4, TC)).astype(E4)

    inv_freq = (1.0 / (THETA ** (np.arange(HALF, dtype=np.float64) / HALF)))
    ang = positions.astype(np.float64)[:, None] * inv_freq[None, :]  # [T, 64]
    cos = np.cos(ang).astype(np.float32).T                           # [64, T]
    sin = np.sin(ang).astype(np.float32).T
    cossin_np = np.ascontiguousarray(np.concatenate(
        [cos, cos, -sin, sin], axis=0).reshape(2, 128, T)
        .transpose(1, 0, 2).reshape(128, 2 * T)).astype(BF)
    tri = (np.arange(128)[None, :] >= np.arange(128)[:, None])       # [dk, dq]
    bmisc_np = np.ascontiguousarray(np.concatenate(
        [np.eye(128, dtype=np.float32), tri.astype(np.float32)],
        axis=1)).astype(BF)
    tri8_np = np.ascontiguousarray(tri.astype(np.float32)).astype(E4)

    in_maps = []
    for c in range(N_CORES):
        wqc = wq[:, c * NH * HD:(c + 1) * NH * HD]
        wkc = wk[:, c * HD:(c + 1) * HD]
        wvc = wv[:, c * HD:(c + 1) * HD]
        woc = np.ascontiguousarray(wo[c * NH * HD:(c + 1) * NH * HD, :])
        wall = np.concatenate([wqc, wkc, wvc], axis=1)       # [H, 768]
        in_maps.append({
            "ht0": ht0_np,
            "ht8": ht8_np,
            "w0all": np.ascontiguousarray(
                wall.reshape(NKCH // 2, 2, 128, WALL)
                .transpose(0, 2, 1, 3)).astype(BF),
            "w8all": np.ascontiguousarray(
                wall.reshape(NPCH, 2, 128, WALL)
                .transpose(0, 2, 1, 3)).astype(E4),
            "wob": woc.astype(BF),
            # [pr, ko, ki, mo] -> [pr, ki, ko, mo]
            "wo8": np.ascontiguousarray(
                woc.reshape(NH // 2, 2, 128, H)
                .transpose(0, 2, 1, 3)).astype(E4),
            "cossin": cossin_np,
            "bmisc": bmisc_np,
            "tri8": tri8_np,
            "ones_r": np.ones((1, 128), dtype=np.float32),
        })

    if _NC is None:
        _NC = _build()
    if _trace:
        _ensure_profile_hook()
    res = bass_utils.run_bass_kernel_spmd(
        _NC, in_maps, core_ids=list(range(N_CORES)), trace=_trace)
    if _trace:
        LAST_EXEC_NS = res.exec_time_ns
        LAST_TRACE = (res.instructions_and_trace[1]
                      if res.instructions_and_trace else None)
        LAST_INSTS = (res.instructions_and_trace[0]
                      if res.instructions_and_trace else None)

    acc = res.results[0]["out_t"].astype(np.float32)
    for c in range(1, N_CORES):
        acc += res.results[c]["out_t"].astype(np.float32)
    # [qc, ki, mo*TC+t] -> [mo*128+ki, qc*TC+t] = [H, T] -> [T, H]
    out_ht = (acc.reshape(NTC, 128, H // 128, TC).transpose(2, 1, 0, 3)
              .reshape(H, T))
    return np.ascontiguousarray(out_ht.T).astype(np.float32)


# revision 44
# speedup vs baseline: 1.1998x; 1.1998x over previous
"""Llama GQA attention layer (T=2048, H=4096, 32 q heads / 8 kv heads, hd=128),
tensor-parallel over heads across 8 Trainium2 NeuronCores.

Per core c: 4 q heads + 1 kv head (wq/wk/wv column slices, wo row slice).
Each core computes a full [T, H] partial o_proj output (bf16); partials are
summed on host (the all-reduce of the TP scheme).

Mixed precision scheme (error budget is dominated by the early causal rows,
which have near-delta attention and O(1)-magnitude outputs):
- t-chunk 0 of QKV, and q-chunk 0 of attention+o_proj run in bf16.
- everything else runs fp8e4(m3) with MatmulPerfMode.DoubleRow (2 fp8
  weights/PE cell, 2 MACs/cycle, measured same 216ns/MM as bf16 at half
  the instruction count): QKV for t-chunks 1-3 contracts hidden-dim pairs,
  PV+softmax-denominator and o_proj for q-chunks 1-3 contract
  key-chunk/head pairs. Those rows average >500 keys, so fp8 noise washes
  out by ~1/sqrt(k_eff) against a 40x larger error budget.
- exp() carries a built-in bias of -ln(16) so probabilities fit fp8e4
  range; the factor cancels between numerator and denominator.

DMA discipline: every dma_start costs ~600ns on the issuing engine queue
(HWDGE descriptor generation), so transfers are batched aggressively:
weights land as one combined [128, 2, 768] (q|k|v) tile per hidden
pair-chunk, hiddenT tiles are loaded two chunks per DMA, RoPE half-swaps
move all 5 heads in two DMAs, and o_proj stores 4 output chunks per DMA.
"""

import sys

if "/opt/trn_rl_repo" not in sys.path:
    sys.path.insert(0, "/opt/trn_rl_repo")

import numpy as np

import concourse.bass as bass
import concourse.bacc as bacc
import concourse.tile as tile
import concourse.mybir as mybir
from concourse import bass_utils

T = 2048
H = 4096
NQ = 32
NKV = 8
HD = 128
THETA = 10000.0
N_CORES = 8
NH = NQ // N_CORES          # local q heads per core
HALF = HD // 2
TC = 512                    # t-chunk (matmul free dim)
NTC = T // TC               # 4
NKCH = H // 128             # 32 hidden chunks
NPCH = H // 256             # 16 hidden pair-chunks (DoubleRow)
WALL = NH * HD + 2 * HD     # 768: combined q|k|v weight columns
SCALE = float(HD) ** -0.5
EXPB = 2.772588722239781    # ln(16): exp bias so fp8 probs stay in range

F32 = mybir.dt.float32
F32R = mybir.dt.float32r
BF16 = mybir.dt.bfloat16
F8 = mybir.dt.float8e4
ALU = mybir.AluOpType
ACTF = mybir.ActivationFunctionType
DR = mybir.MatmulPerfMode.DoubleRow


def _build():
    nc = bacc.Bacc("TRN2", target_bir_lowering=False, debug=False,
                   num_devices=N_CORES)
    # t-chunk 0 hiddenT pair tiles, bf16: [kp][ki, ko, t] = hT[256kp+128ko+ki, t]
    ht0 = nc.dram_tensor("ht0", [NPCH, 128, 2, TC], BF16,
                         kind="ExternalInput").ap()
    # t-chunks 1-3 hiddenT double-pair tiles, fp8:
    # [(t-1)*8+pq][ki, 2*(pc%2)+ko, t'] = hT[256pc+128ko+ki, 512t+t']
    ht8 = nc.dram_tensor("ht8", [3 * NPCH // 2, 128, 4, TC], F8,
                         kind="ExternalInput").ap()
    # combined bf16 weight pairs [kp][ki, ko, q0..q3|k|v]
    w0all = nc.dram_tensor("w0all", [NKCH // 2, 128, 2, WALL], BF16,
                           kind="ExternalInput").ap()
    # combined fp8 pair weights [pc][ki, ko, q0..q3|k|v]
    w8all = nc.dram_tensor("w8all", [NPCH, 128, 2, WALL], F8,
                           kind="ExternalInput").ap()
    wob = nc.dram_tensor("wob", [NH * HD, H], BF16, kind="ExternalInput").ap()
    wo8 = nc.dram_tensor("wo8", [NH // 2, 128, 2, H], F8,
                         kind="ExternalInput").ap()
    # cos | sin, sign-folded, bf16
    cossin = nc.dram_tensor("cossin", [HD, 2 * T], BF16,
                            kind="ExternalInput").ap()
    # identity | causal-triangle, bf16
    bmisc = nc.dram_tensor("bmisc", [128, 256], BF16,
                           kind="ExternalInput").ap()
    tri8 = nc.dram_tensor("tri8", [128, 128], F8, kind="ExternalInput").ap()
    ones_r = nc.dram_tensor("ones_r", [1, 128], F32R,
                            kind="ExternalInput").ap()
    # per q-chunk: [qc][ki, mo*TC + t']
    out_t = nc.dram_tensor("out_t", [NTC, 128, (H // 128) * TC], BF16,
                           kind="ExternalOutput").ap()

    with tile.TileContext(nc) as tc:
        _body(tc, ht0, ht8, w0all, w8all, wob, wo8, cossin, bmisc, tri8,
              ones_r, out_t)
    nc.compile()
    return nc


def _body(tc, ht0, ht8, w0all, w8all, wob, wo8, cossin, bmisc, tri8,
          ones_r, out_t):
    nc = tc.nc

    with (
        tc.tile_pool(name="persist", bufs=1) as persist,
        tc.tile_pool(name="small", bufs=1) as small,
        tc.tile_pool(name="p0p", bufs=NH * 4) as p0p,
    ):
        # live across all phases
        qT = persist.tile([128, NH * T], BF16, tag="qT")     # [d, h*T + t]
        kT = persist.tile([128, T], BF16, tag="kT")          # [d, t]
        v_sb = persist.tile([128, T], BF16, tag="v")         # chunk i: v[t,d]
        v8 = persist.tile([128, NTC * 4, 128], F8, tag="v8")  # [t, chunk, d]
        bmisc_sb = small.tile([128, 256], BF16, tag="bmisc")
        tri8_sb = small.tile([128, 128], F8, tag="tri8")
        onesb_sb = small.tile([128, 1], BF16, tag="onesb")
        ones8_sb = small.tile([128, 2, 16], F8, tag="ones8")
        onesr_sb = small.tile([1, 128], F32R, tag="onesr")
        ebias_sb = small.tile([128, 1], F32, tag="ebias")
        nc.vector.memset(onesb_sb[:], 1.0)
        nc.vector.memset(ones8_sb[:], 1.0)
        nc.vector.memset(ebias_sb[:], -EXPB)
        ident_sb = bmisc_sb[:, 0:128]
        trib_sb = bmisc_sb[:, 128:256]

        qc0_ps = []  # q-chunk 0 probabilities, produced during phase 1
        # ---------------- phase 1: QKV projections + RoPE + V transpose ----
        with (
            tc.tile_pool(name="ph1w", bufs=1) as ph1w,
            tc.tile_pool(name="ph1", bufs=6) as ph1,
            tc.tile_pool(name="rope", bufs=2) as rope,
            tc.tile_pool(name="ps1", bufs=1, space="PSUM") as ps1,
            tc.tile_pool(name="pst", bufs=1, space="PSUM") as pst,
            tc.tile_pool(name="ps_s0", bufs=1, space="PSUM") as ps_s0,
        ):
            csb = ph1w.tile([128, 2 * T], BF16, tag="csb")
            vT_sb = ph1w.tile([128, T], BF16, tag="vT")
            w0_t, w8_t = [], []

            def _transp(t):
                # V has no rope; transpose vT to natural [t, d] + fp8 copy
                for i in range(t * 4, t * 4 + 4):
                    tp = pst.tile([128, 128], BF16, tag="tp")
                    nc.tensor.transpose(
                        tp[:], vT_sb[:, i * 128:(i + 1) * 128], ident_sb)
                    nc.scalar.copy(v_sb[:, i * 128:(i + 1) * 128], tp[:])
                    nc.vector.tensor_copy(v8[:, i, :],
                                          v_sb[:, i * 128:(i + 1) * 128])

            def _rope(t, qps, kps, vps):
                # RoPE: out = x*cos2 + swap(x)*sin2 (swap = halves exchanged).
                # k first, then head 0, then v, so q-chunk 0's attention can
                # start as early as possible after the t=0 pass.
                raw5 = rope.tile([128, 5 * TC], BF16, tag="raw5")
                sw5 = rope.tile([128, 5 * TC], BF16, tag="sw5")
                cosv = csb[:, t * TC:(t + 1) * TC]
                sinv = csb[:, T + t * TC: T + (t + 1) * TC]
                for step, hc in enumerate((NH, 0, 1, 2, 3)):
                    src_ps = qps[hc] if hc < NH else kps
                    sl = slice(hc * TC, (hc + 1) * TC)
                    nc.scalar.copy(raw5[:, sl], src_ps[:])
                    # gpsimd SWDGE keeps the half-swaps off the Sync queue
                    # so the next t-chunk's ht/weight loads aren't blocked
                    nc.gpsimd.dma_start(sw5[0:HALF, sl], raw5[HALF:128, sl])
                    nc.gpsimd.dma_start(sw5[HALF:128, sl], raw5[0:HALF, sl])
                    dst = (qT[:, hc * T + t * TC: hc * T + (t + 1) * TC]
                           if hc < NH else kT[:, t * TC:(t + 1) * TC])
                    a = rope.tile([128, TC], BF16, tag="ra")
                    b = rope.tile([128, TC], BF16, tag="rb16")
                    nc.vector.tensor_tensor(a[:], raw5[:, sl], cosv, ALU.mult)
                    nc.vector.tensor_tensor(b[:], sw5[:, sl], sinv, ALU.mult)
                    nc.vector.tensor_tensor(dst, a[:], b[:], ALU.add)
                    if step == 1:
                        nc.scalar.copy(vT_sb[:, t * TC:(t + 1) * TC], vps[:])
                        _transp(t)

            def _qc0_scores():
                # q-chunk 0 scores+exp overlap the t=1..3 QKV stream: they
                # only need t-chunk 0's rope output, and the exp chain hides
                # under ~60us of DoubleRow matmuls instead of padding phase 2
                for h in range(NH):
                    qh = qT[:, h * T: h * T + TC]
                    for kc in range(4):
                        off = 128 * kc      # all qc=0 chunks diagonal
                        sT = ps_s0.tile([128, TC], F32, tag="sT0")
                        nc.tensor.matmul(
                            sT[:, off:], kT[:, kc * 128:(kc + 1) * 128],
                            qh[:, off:], start=True, stop=True)
                        p = p0p.tile([128, TC], BF16, tag="p0", name="p0")
                        nc.scalar.activation(p[:, off:], sT[:, off:],
                                             ACTF.Exp, scale=SCALE,
                                             bias=ebias_sb[:])
                        if off:
                            nc.gpsimd.memset(p[:, 0:off], 0.0)
                        nc.gpsimd.tensor_tensor(
                            p[:, off:off + 128], p[:, off:off + 128],
                            trib_sb, ALU.mult)
                        qc0_ps.append(p)

            for t in (0, 1, 2, 3):
                qps = [ps1.tile([128, TC], F32, tag=f"qps{fc}",
                                name=f"qps{fc}") for fc in range(NH)]
                kps = ps1.tile([128, TC], F32, tag="kps")
                vps = ps1.tile([128, TC], F32, tag="vps")
                if t == 0:
                    for k in range(NKCH):
                        if k % 2 == 0:
                            w0t = ph1w.tile([128, 2, WALL], BF16,
                                            tag=f"w0a{k // 2}",
                                            name=f"w0a{k // 2}")
                            if k == 0:
                                # k/v slice first: the opening matmul only
                                # waits on 64KB instead of 384KB
                                nc.sync.dma_start(w0t[:, 0, 512:768],
                                                  w0all[0][:, 0, 512:768])
                                nc.sync.dma_start(w0t[:, 0, 0:512],
                                                  w0all[0][:, 0, 0:512])
                                nc.sync.dma_start(w0t[:, 1, :],
                                                  w0all[0][:, 1, :])
                            else:
                                nc.sync.dma_start(w0t[:], w0all[k // 2])
                            w0_t.append(w0t)
                        if k == 4:
                            nc.sync.dma_start(bmisc_sb[:], bmisc[:, :])
                        if k == 5:
                            nc.sync.dma_start(tri8_sb[:], tri8[:, :])
                        if k == 8:
                            nc.sync.dma_start(csb[:], cossin[:, :])
                        if k % 2 == 0:
                            htt = ph1.tile([128, 2, TC], BF16, tag="ht0")
                            if k == 0:
                                # split the very first load so MM #1 only
                                # waits on half the bytes
                                nc.sync.dma_start(htt[:, 0, :],
                                                  ht0[0][:, 0, :])
                                nc.sync.dma_start(htt[:, 1, :],
                                                  ht0[0][:, 1, :])
                            else:
                                nc.sync.dma_start(htt[:], ht0[k // 2])
                        mv = htt[:, k % 2, :]
                        st, sp = (k == 0), (k == NKCH - 1)
                        wsl = w0_t[k // 2]
                        # k/v first: kps completes earlier, shortening the
                        # RoPE-k chain that gates q-chunk 0's first scores
                        nc.tensor.matmul(kps[:], wsl[:, k % 2, 512:640], mv,
                                         start=st, stop=sp)
                        nc.tensor.matmul(vps[:], wsl[:, k % 2, 640:768], mv,
                                         start=st, stop=sp)
                        for fc in range(NH):
                            nc.tensor.matmul(
                                qps[fc][:],
                                wsl[:, k % 2, fc * 128:(fc + 1) * 128],
                                mv, start=st, stop=sp)
                else:
                    for pc in range(NPCH):
                        if t == 1:
                            w8t = ph1w.tile([128, 2, WALL], F8,
                                            tag=f"w8a{pc}", name=f"w8a{pc}")
                            nc.sync.dma_start(w8t[:], w8all[pc])
                            w8_t.append(w8t)
                        if pc % 2 == 0:
                            htt = ph1.tile([128, 4, TC], F8, tag="ht8")
                            nc.sync.dma_start(
                                htt[:], ht8[(t - 1) * (NPCH // 2) + pc // 2])
                        mv = htt[:, 2 * (pc % 2):2 * (pc % 2) + 2, :]
                        st, sp = (pc == 0), (pc == NPCH - 1)
                        for fc in range(NH):
                            nc.tensor.matmul(
                                qps[fc][:],
                                w8_t[pc][:, :, fc * 128:(fc + 1) * 128],
                                mv, start=st, stop=sp, perf_mode=DR)
                        nc.tensor.matmul(
                            kps[:], w8_t[pc][:, :, 512:640], mv,
                            start=st, stop=sp, perf_mode=DR)
                        nc.tensor.matmul(
                            vps[:], w8_t[pc][:, :, 640:768], mv,
                            start=st, stop=sp, perf_mode=DR)
                _rope(t, qps, kps, vps)
                if t == 0:
                    _qc0_scores()

        # ------- phase 2+3: attention interleaved with o_proj, per q-chunk --
        with (
            tc.tile_pool(name="late", bufs=1) as late,
            tc.tile_pool(name="attq", bufs=2) as attq,
            tc.tile_pool(name="att8", bufs=2) as att8,
            tc.tile_pool(name="p8p", bufs=6) as p8p,
            tc.tile_pool(name="pbp", bufs=4) as pbp,
            tc.tile_pool(name="obp", bufs=4) as obp,
            tc.tile_pool(name="rbp", bufs=8) as rbp,
            tc.tile_pool(name="ps_s", bufs=3, space="PSUM") as ps_s,
            tc.tile_pool(name="ps_o", bufs=1, space="PSUM") as ps_o,
            tc.tile_pool(name="ps_d", bufs=1, space="PSUM") as ps_d,
            tc.tile_pool(name="ps_op", bufs=2, space="PSUM") as ps_op,
            tc.tile_pool(name="ps_rb", bufs=1, space="PSUM") as ps_rb,
        ):
            attn_n = late.tile([128, NH * TC], BF16, tag="attn_n")
            nc.sync.dma_start(onesr_sb[:], ones_r[:, :])
            wob_t, wo8_t = [], []
            for h in range(NH):
                wot = late.tile([128, H], BF16, tag=f"wob{h}", name=f"wob{h}")
                nc.sync.dma_start(wot[:], wob[h * 128:(h + 1) * 128, :])
                wob_t.append(wot)
            for pr in range(NH // 2):
                wo8t = late.tile([128, 2, H], F8, tag=f"wo8{pr}",
                                 name=f"wo8{pr}")
                nc.sync.dma_start(wo8t[:], wo8[pr])
                wo8_t.append(wo8t)

            def _norm(qc, attn_t, rcs):
                """normalize by 1/den, broadcast [1,TC] across partitions via
                a rank-1 PE matmul (ones_col x recip_row) into PSUM; write
                bf16 (qc=0) or fp8 head-slabs (qc>=1) for o_proj."""
                a8 = None
                if qc > 0:
                    a8 = att8.tile([128, NH, TC], F8, tag="attn8",
                                   name="attn8")
                for h, rcr in enumerate(rcs):
                    rb = ps_rb.tile([128, TC], F32, tag="rbps", name="rbps")
                    nc.tensor.matmul(rb[:], onesr_sb[:], rcr[:],
                                     start=True, stop=True)
                    dst = (attn_n[:, h * TC:(h + 1) * TC] if qc == 0
                           else a8[:, h, :])
                    nc.vector.tensor_tensor(
                        dst, attn_t[:, h * TC:(h + 1) * TC], rb[:], ALU.mult)
                return a8

            def _oproj(qc, a8):
                """out_t[qc][mo-part, mo*TC+t] = sum_f wo[f, mo] * attnT[f, t];
                4 mo-chunks per store DMA, drains alternate Scalar/Vector."""
                ob4 = None
                for mo in range(H // 128):
                    op = ps_op.tile([128, TC], F32, tag="op", name="op")
                    if qc == 0:
                        for h in range(NH):
                            nc.tensor.matmul(
                                op[:],
                                wob_t[h][:, mo * 128:(mo + 1) * 128],
                                attn_n[:, h * TC:(h + 1) * TC],
                                start=(h == 0), stop=(h == NH - 1))
                    else:
                        for pr in range(NH // 2):
                            nc.tensor.matmul(
                                op[:],
                                wo8_t[pr][:, :, mo * 128:(mo + 1) * 128],
                                a8[:, 2 * pr:2 * pr + 2, :],
                                start=(pr == 0), stop=(pr == NH // 2 - 1),
                                perf_mode=DR)
                    if mo % 4 == 0:
                        ob4 = obp.tile([128, 4 * TC], BF16, tag="ob4",
                                       name="ob4")
                    dst = ob4[:, (mo % 4) * TC:(mo % 4 + 1) * TC]
                    nc.vector.tensor_copy(dst, op[:])
                    if mo % 4 == 3:
                        nc.sync.dma_start(
                            out_t[qc][:, (mo - 3) * TC:(mo + 1) * TC], ob4[:])

            norm_pending = []
            for qc in range(NTC):
                nkc = (qc + 1) * (TC // 128)     # causal k chunks of 128
                attn_t = attq.tile([128, NH * TC], BF16, tag="attnq",
                                   name="attnq")
                rcs = []
                for h in range(NH):
                    qh = qT[:, h * T + qc * TC: h * T + (qc + 1) * TC]
                    po = ps_o.tile([128, TC], F32, tag="po")
                    pd = ps_d.tile([1, TC], F32, tag="pd")
                    if qc == 0:
                        ps = qc0_ps[h * 4:(h + 1) * 4]

                        def _pv0(kc, nkc=nkc, po=po, pd=pd, ps=ps):
                            st, sp = (kc == 0), (kc == nkc - 1)
                            o = 128 * kc        # masked-zero column prefix
                            nc.tensor.matmul(
                                po[:, o:], v_sb[:, kc * 128:(kc + 1) * 128],
                                ps[kc][:, o:], start=st, stop=sp,
                                skip_group_check=True)
                            nc.tensor.matmul(pd[:, o:], onesb_sb[:],
                                             ps[kc][:, o:],
                                             start=st, stop=sp,
                                             skip_group_check=True)

                        for kc in range(nkc):
                            _pv0(kc)
                    else:
                        npair = nkc // 2
                        prs = []

                        def _pvp(r, npair=npair, nkc=nkc, po=po, pd=pd,
                                 prs=prs):
                            st, sp = (r == 0), (r == npair - 1)
                            # masked-zero column prefix of the even chunk
                            o = max(0, 128 * (2 * r - (nkc - 4)))
                            nc.tensor.matmul(
                                po[:, o:], v8[:, 2 * r:2 * r + 2, :],
                                prs[r][:, :, o:], start=st, stop=sp,
                                perf_mode=DR, skip_group_check=True)
                            nc.tensor.matmul(
                                pd[:, o:], ones8_sb[:, :, 0:1],
                                prs[r][:, :, o:], start=st, stop=sp,
                                perf_mode=DR, skip_group_check=True)

                        for kc in range(nkc):
                            di = kc - (nkc - 4)
                            off = 128 * di if di >= 0 else 0
                            slab = kc & 1
                            if slab == 0:
                                p8t = p8p.tile([128, 2, TC], F8, tag="p8",
                                               name="p8")
                                prs.append(p8t)
                            else:
                                p8t = prs[-1]
                            sT = ps_s.tile([128, TC], F32, tag="sT")
                            nc.tensor.matmul(
                                sT[:, off:], kT[:, kc * 128:(kc + 1) * 128],
                                qh[:, off:], start=True, stop=True)
                            nc.scalar.activation(p8t[:, slab, off:],
                                                 sT[:, off:], ACTF.Exp,
                                                 scale=SCALE,
                                                 bias=ebias_sb[:])
                            if off:
                                nc.gpsimd.memset(p8t[:, slab, 0:off], 0.0)
                            if di >= 0:
                                nc.gpsimd.tensor_tensor(
                                    p8t[:, slab, off:off + 128],
                                    p8t[:, slab, off:off + 128],
                                    tri8_sb[:], ALU.mult)
                            # PV trails the score stream by one pair so the
                            # PE never waits on the exp
                            if slab == 1 and len(prs) >= 2:
                                _pvp(len(prs) - 2)
                        _pvp(npair - 1)
                    # drain PSUM fast; normalization happens in _norm, off
                    # the PE critical path
                    nc.vector.tensor_copy(
                        attn_t[:, h * TC:(h + 1) * TC], po[:])
                    rc = rbp.tile([1, TC], F32, tag="rc")
                    nc.vector.reciprocal_approx_fast(out=rc[:], in_=pd[:])
                    rcr = rbp.tile([1, TC], F32R, tag="rcr")
                    nc.vector.tensor_copy(rcr[:], rc[:])
                    rcs.append(rcr)
                norm_pending.append((qc, attn_t, rcs))

                # normalize + o_proj pipelined one q-chunk behind attention so
                # the reciprocal chain never stalls the PE stream
                if qc >= 1:
                    pqc, pattn, prcs = norm_pending.pop(0)
                    a8 = _norm(pqc, pattn, prcs)
                    _oproj(pqc, a8)
            pqc, pattn, prcs = norm_pending.pop(0)
            a8 = _norm(pqc, pattn, prcs)
            _oproj(pqc, a8)


_NC = None
LAST_EXEC_NS = None
LAST_TRACE = None
LAST_INSTS = None


def _ensure_profile_hook():
    """Register the axon NTFF profiling hook (container lacks antenv.axon_hooks)."""
    import types
    import antenv
    if "antenv.axon_hooks" in sys.modules:
        return
    hooks_mod = types.ModuleType("antenv.axon_hooks")
    _h = [None]
    hooks_mod.set_axon_ntff_profile_hook = lambda hk: _h.__setitem__(0, hk)
    hooks_mod.get_axon_ntff_profile_hook = lambda: _h[0]
    sys.modules["antenv.axon_hooks"] = hooks_mod
    antenv.axon_hooks = hooks_mod
    from trn_agent_boot.trn_boot import _ntff_profile_via_ctypes
    hooks_mod.set_axon_ntff_profile_hook(
        _ntff_profile_via_ctypes("/opt/axon/libaxon_pjrt.so"))
    bass_utils.upload_artifacts = lambda tmpdir: "local://skipped"


def kernel(positions, hidden_states, wq, wk, wv, wo, _trace=False, **_unused):
    global _NC, LAST_EXEC_NS, LAST_TRACE, LAST_INSTS
    import ml_dtypes
    BF = ml_dtypes.bfloat16
    E4 = ml_dtypes.float8_e4m3

    positions = np.asarray(positions)
    hidden_states = np.asarray(hidden_states, dtype=np.float32)
    wq = np.asarray(wq, dtype=np.float32)
    wk = np.asarray(wk, dtype=np.float32)
    wv = np.asarray(wv, dtype=np.float32)
    wo = np.asarray(wo, dtype=np.float32)

    # host-side input prep (sharding + layout)
    hT = np.ascontiguousarray(hidden_states.T)                      # [H, T]
    # [kp, ko, ki, t] from hT[256kp + 128ko + ki, t]
    ht0_np = np.ascontiguousarray(
        hT[:, 0:TC].reshape(NPCH, 2, 128, TC)
        .transpose(0, 2, 1, 3)).astype(BF)
    # [pc, ko, ki, tc, t] -> [tc, pc, ki, ko, t], tc 1..3, then group pc pairs
    x8 = (hT.reshape(NPCH, 2, 128, NTC, TC).transpose(3, 0, 2, 1, 4)[1:]
          .reshape(3, NPCH // 2, 2, 128, 2, TC).transpose(0, 1, 3, 2, 4, 5))
    ht8_np = np.ascontiguousarray(
        x8.reshape(3 * NPCH // 2, 128, 4, TC)).astype(E4)

    inv_freq = (1.0 / (THETA ** (np.arange(HALF, dtype=np.float64) / HALF)))
    ang = positions.astype(np.float64)[:, None] * inv_freq[None, :]  # [T, 64]
    cos = np.cos(ang).astype(np.float32).T                           # [64, T]
    sin = np.sin(ang).astype(np.float32).T
    cossin_np = np.ascontiguousarray(np.concatenate(
        [cos, cos, -sin, sin], axis=0).reshape(2, 128, T)
        .transpose(1, 0, 2).reshape(128, 2 * T)).astype(BF)
    tri = (np.arange(128)[None, :] >= np.arange(128)[:, None])       # [dk, dq]
    bmisc_np = np.ascontiguousarray(np.concatenate(
        [np.eye(128, dtype=np.float32), tri.astype(np.float32)],
        axis=1)).astype(BF)
    tri8_np = np.ascontiguousarray(tri.astype(np.float32)).astype(E4)

    in_maps = []
    for c in range(N_CORES):
        wqc = wq[:, c * NH * HD:(c + 1) * NH * HD]
        wkc = wk[:, c * HD:(c + 1) * HD]
        wvc = wv[:, c * HD:(c + 1) * HD]
        woc = np.ascontiguousarray(wo[c * NH * HD:(c + 1) * NH * HD, :])
        wall = np.concatenate([wqc, wkc, wvc], axis=1)       # [H, 768]
        in_maps.append({
            "ht0": ht0_np,
            "ht8": ht8_np,
            "w0all": np.ascontiguousarray(
                wall.reshape(NKCH // 2, 2, 128, WALL)
                .transpose(0, 2, 1, 3)).astype(BF),
            "w8all": np.ascontiguousarray(
                wall.reshape(NPCH, 2, 128, WALL)
                .transpose(0, 2, 1, 3)).astype(E4),
            "wob": woc.astype(BF),
            # [pr, ko, ki, mo] -> [pr, ki, ko, mo]
            "wo8": np.ascontiguousarray(
                woc.reshape(NH // 2, 2, 128, H)
                .transpose(0, 2, 1, 3)).astype(E4),
            "cossin": cossin_np,
            "bmisc": bmisc_np,
            "tri8": tri8_np,
            "ones_r": np.ones((1, 128), dtype=np.float32),
        })

    if _NC is None:
        _NC = _build()
    if _trace:
        _ensure_profile_hook()
    res = bass_utils.run_bass_kernel_spmd(
        _NC, in_maps, core_ids=list(range(N_CORES)), trace=_trace)
    if _trace:
        LAST_EXEC_NS = res.exec_time_ns
        LAST_TRACE = (res.instructions_and_trace[1]
                      if res.instructions_and_trace else None)
        LAST_INSTS = (res.instructions_and_trace[0]
                      if res.instructions_and_trace else None)

    acc = res.results[0]["out_t"].astype(np.float32)
    for c in range(1, N_CORES):
        acc += res.results[c]["out_t"].astype(np.float32)
    # [qc, ki, mo*TC+t] -> [mo*128+ki, qc*TC+t] = [H, T] -> [T, H]
    out_ht = (acc.reshape(NTC, 128, H // 128, TC).transpose(2, 1, 0, 3)
              .reshape(H, T))
    return np.ascontiguousarray(out_ht.T).astype(np.float32)
